# revision 17
# baseline (speedup 1.0000x reference)
"""Trainium2 Bass kernel for GQA attention block (nn_Attention_6219112644965).

Reference computation (per batch b):
  q = rope(rmsnorm(x @ Wq, q_gamma), cos, sin)   # 16 heads x 128
  k = rope(rmsnorm(x @ Wk, k_gamma), cos, sin)   # 8 kv heads x 128
  v = x @ Wv
  o = softmax(q k^T / sqrt(128)) v               # GQA: q head h uses kv head h//2
  y = o @ Wo
Sharding: 8 cores = 4 batches x 2 head-groups; host sums the two Wo-partials
per batch.  Head-dim-major on-chip layout (no transposes anywhere) -- see the
packing code at the bottom.

Schedule (v2).  All engine roles:
  PE    : the six GEMM stages only (no reductions -- those moved to POOL).
  ACT   : exp(scores) in phase 2 (one N=2048 ACTIVATE per PSUM pair-tile,
          amortizing the ~350-cycle fixed cost), rmsnorm Ln/Exp + PSUM
          copies in phase 1.
  DVE   : rope muls, half the colsum accumulation, softmax recip + normalize,
          out-proj PSUM->SBUF copies.
  POOL  : partition reductions (rmsnorm sum-of-squares, softmax denominator
          via partition_all_reduce) + the other half of colsum accumulation.
Phase 1 (k, v, q projections): k heads 0/1 contract d-incrementally across all
8 PSUM banks so the PE tracks the xt DMA stream with no startup stall.
Phase 2 (attention): scores for two consecutive k-tiles land in one
[128, 2, 1024] PSUM pair-tile (4 banks) and are exponentiated by a single
ACTIVATE; o-matmuls lag s-matmuls by LAG slots so exp latency never stalls
the PE; the out-projection for t-tiles 0..7 is woven one LDW-pair unit at a
time into the chunk-1 slots (filling the exp-latency slack), with the np0->np1
PSUM handoff padded two slots so DVE copies hide; t-tiles 8..15 run in a tail
scope with 6 rotating PSUM banks.  Softmax max-subtraction is skipped
(rmsnorm-ed q,k give |s| <~ 6; fp32 exp is exact there); 1/sqrt(HD) and the
rope pair-shuffle sign/gamma are folded into the host-packed tables.
"""
import sys

sys.path.insert(0, "/opt/trn_rl_repo")

from collections import deque
from contextlib import ExitStack

import ml_dtypes
import numpy as np

import bass_rust
import concourse.bass as bass
import concourse.mybir as mybir
import concourse.tile as tile
from concourse import bacc, bass_isa, hw_specs
from concourse.bass_utils import run_bass_kernel_spmd

F32 = mybir.dt.float32
BF16 = mybir.dt.bfloat16
AF = mybir.ActivationFunctionType
RADD = bass_isa.ReduceOp.add

T = 2048          # sequence length
D = 2048          # model dim
HD = 128          # head dim
NQH = 8           # q heads per core
NKV = 4           # kv heads per core
ND = D // 128     # 16 d-tiles
NTT = T // 128    # 16 t-tiles
TQC = 1024        # tq chunk
LAG = 4           # o-matmul lag behind s-matmul, in slots
EPS = 1e-6

_CACHE = {}
LAST_RESULTS = None


class _Bacc(bacc.Bacc):
    """Bacc with Exp pinned to the natural_log_exp_and_others ACT table set.

    The default static func->set assignment maps Exp to `exp_and_others`
    and Ln to `natural_log_exp_and_others`; a kernel alternating Ln and Exp
    then reloads the ACT tables (~2.7us) on every transition.  Hiding `exp`
    from the other sets makes both resolve to the shared set, so the table
    is loaded once for the whole kernel.
    """

    def insert_act_table_loads(self):
        has_activation = any(
            isinstance(i, mybir.InstActivation)
            for b in self.main_func.blocks
            for i in b.instructions
        )
        if not has_activation:
            return
        tables = []
        for name, funcs in hw_specs.get_activation_tables(self.m.arch).items():
            if name != "natural_log_exp_and_others":
                funcs = funcs - {AF.Exp}
            tables.append((name, funcs))
        bass_rust.insert_act_table_loads(self, tables)


def build_module():
    """Build the per-core Bass program (identical on all 8 cores)."""
    nc = _Bacc("TRN2", target_bir_lowering=False, debug=False)

    # ---- DRAM I/O (host-packed so every DMA is contiguous) ----
    xt_d = nc.dram_tensor("xt", [128, ND, T], BF16, kind="ExternalInput")
    wq_d = nc.dram_tensor("wq", [NQH, 128, ND, HD], BF16, kind="ExternalInput")
    wk_d = nc.dram_tensor("wk", [NKV, 128, ND, HD], BF16, kind="ExternalInput")
    wv_d = nc.dram_tensor("wv", [128, ND, NKV * HD], BF16, kind="ExternalInput")
    wo_d = nc.dram_tensor("wo", [128, NQH, D], BF16, kind="ExternalInput")
    cosq_d = nc.dram_tensor("cosq", [128, T], BF16, kind="ExternalInput")
    sinq_d = nc.dram_tensor("sinq", [128, T], BF16, kind="ExternalInput")
    cosk_d = nc.dram_tensor("cosk", [128, T], BF16, kind="ExternalInput")
    sink_d = nc.dram_tensor("sink", [128, T], BF16, kind="ExternalInput")
    y_d = nc.dram_tensor("y", [T, D], F32, kind="ExternalOutput")

    with tile.TileContext(nc) as tc, ExitStack() as top:
        persist = top.enter_context(tc.tile_pool(name="persist", bufs=1))
        qT = persist.tile([128, NQH, T], BF16, tag="qT")     # q^T, rope+norm done
        kT = persist.tile([128, NKV, T], BF16, tag="kT")     # k^T, rope+norm done
        v_sb = persist.tile([128, NTT, NKV * HD], BF16, tag="v")  # v natural
        zero128 = persist.tile([128, 1], F32, tag="zero128")
        nc.vector.memset(zero128, 0.0)
        ones_col = persist.tile([128, 1], BF16, tag="ones")
        nc.vector.memset(ones_col, 1.0)
        epsq1 = persist.tile([1, 1], F32, tag="epsq")
        nc.vector.memset(epsq1, float(HD * EPS))
        epsk1 = persist.tile([1, 1], F32, tag="epsk")
        nc.vector.memset(epsk1, float(EPS))
        zero1 = zero128[0:1, :]

        # ================= phase 1: projections =================
        with ExitStack() as ph1:
            p1 = ph1.enter_context(tc.tile_pool(name="p1", bufs=1))
            xt = p1.tile([128, ND, T], BF16, tag="xt")
            wv_sb = p1.tile([128, ND, NKV * HD], BF16, tag="wv")
            cosq = p1.tile([128, T], BF16, tag="cosq")
            sinq = p1.tile([128, T], BF16, tag="sinq")
            cosk = p1.tile([128, T], BF16, tag="cosk")
            sink = p1.tile([128, T], BF16, tag="sink")

            wslab_p = ph1.enter_context(tc.tile_pool(name="wslab", bufs=3))
            work = ph1.enter_context(tc.tile_pool(name="pwork", bufs=3))
            rows = ph1.enter_context(tc.tile_pool(name="rows", bufs=2))
            mwork = ph1.enter_context(tc.tile_pool(name="mwork", bufs=3))
            ps_mm = ph1.enter_context(tc.tile_pool(name="ps_mm", bufs=7, space="PSUM"))
            ps_row = ph1.enter_context(tc.tile_pool(name="ps_row", bufs=1, space="PSUM"))

            # -- DMA issue order == arrival order: k0/k1 slabs gate the first
            # matmuls, xt d-tiles pace the incremental contraction, then the
            # k tables, the k2/k3 slabs, wv, and the q tables.
            wsl_k = []
            for h in (0, 1):
                wsl = wslab_p.tile([128, ND, HD], BF16, tag="wsl")
                nc.sync.dma_start(out=wsl[:], in_=wk_d[h])
                wsl_k.append(wsl)
            for d in range(ND):
                nc.sync.dma_start(out=xt[:, d, :], in_=xt_d[:, d, :])
            nc.sync.dma_start(out=cosk[:], in_=cosk_d[:])
            nc.sync.dma_start(out=sink[:], in_=sink_d[:])
            for h in (2, 3):
                wsl = wslab_p.tile([128, ND, HD], BF16, tag="wsl")
                nc.sync.dma_start(out=wsl[:], in_=wk_d[h])
                wsl_k.append(wsl)
            nc.sync.dma_start(out=wv_sb[:], in_=wv_d[:])
            nc.sync.dma_start(out=cosq[:], in_=cosq_d[:])
            nc.sync.dma_start(out=sinq[:], in_=sinq_d[:])

            def chain_pre(raw_ps):
                """PSUM consumers of one projection chunk: bf16 copy + square.

                Emitted right after the chunk's matmuls; the partition
                reduction and everything after it is deferred one chunk (see
                chain_post) so the ones-matmul never waits on the Square.
                """
                raw_bf = work.tile([128, 512], BF16, tag="raw")
                nc.scalar.copy(raw_bf, raw_ps)
                sq = work.tile([128, 512], BF16, tag="sq")
                nc.scalar.activation(out=sq, in_=raw_ps, func=AF.Square,
                                     bias=zero128[:, :])
                return raw_bf, sq

            def chain_post(raw_bf, sq, out_T, h, c, cos_t, sin_t, is_q):
                """rmsnorm reduction + rope for one chunk (deferred).

                rec = (ssq*scale+eps)^-1/2 via exp(-0.5*ln(.)), one shared
                ACT table set; the q-side folds the extra 1/sqrt(HD) by
                using scale=1, eps=HD*eps.
                """
                cs = slice(c * 512, (c + 1) * 512)
                ssq = ps_row.tile([1, 512], F32, tag="row")
                nc.tensor.matmul(ssq, ones_col, sq, start=True, stop=True)
                lnrow = rows.tile([1, 512], F32, tag="lnrow")
                if is_q:
                    nc.scalar.activation(out=lnrow, in_=ssq, func=AF.Ln,
                                         scale=1.0, bias=epsq1[:, :])
                else:
                    nc.scalar.activation(out=lnrow, in_=ssq, func=AF.Ln,
                                         scale=1.0 / HD, bias=epsk1[:, :])
                rec = rows.tile([1, 512], F32, tag="recrow")
                nc.scalar.activation(out=rec, in_=lnrow, func=AF.Exp, scale=-0.5,
                                     bias=zero1)
                bc = work.tile([128, 512], F32, tag="bc")
                nc.gpsimd.partition_broadcast(bc, rec)
                # rope: out = (raw*cos + swap64(raw)*sin) * bc
                m1 = mwork.tile([128, 512], BF16, tag="m1")
                nc.vector.tensor_mul(m1, raw_bf, cos_t[:, cs])
                swp = mwork.tile([128, 512], BF16, tag="swp")
                nc.vector.tensor_copy(swp[0:64, :], raw_bf[64:128, :])
                nc.vector.tensor_copy(swp[64:128, :], raw_bf[0:64, :])
                m2 = mwork.tile([128, 512], BF16, tag="m2")
                nc.vector.tensor_mul(m2, swp, sin_t[:, cs])
                m3 = mwork.tile([128, 512], BF16, tag="m3")
                nc.vector.tensor_add(m3, m1, m2)
                nc.vector.tensor_mul(out_T[:, h, cs], m3, bc)

            pending = []

            def defer_chain(raw_ps, out_T, h, c, cos_t, sin_t, is_q):
                raw_bf, sq = chain_pre(raw_ps)
                pending.append(lambda: chain_post(raw_bf, sq, out_T, h, c,
                                                  cos_t, sin_t, is_q))

            def emit_pending(n=1):
                for _ in range(min(n, len(pending))):
                    pending.pop(0)()

            # -- k heads 0/1: contract d-incrementally across 7 PSUM banks so
            # the PE keeps pace with the xt DMA stream from the first tile.
            inc = [(0, 0), (0, 1), (0, 2), (0, 3), (1, 0), (1, 1), (1, 2)]
            accs = [ps_mm.tile([128, 512], F32, tag="mm", name=f"acc{i}")
                    for i in range(len(inc))]
            for d in range(ND):
                for i, (h, c) in enumerate(inc):
                    nc.tensor.matmul(accs[i], wsl_k[h][:, d, :],
                                     xt[:, d, c * 512:(c + 1) * 512],
                                     start=(d == 0), stop=(d == ND - 1))
            for i, (h, c) in enumerate(inc):
                defer_chain(accs[i], kT, h, c, cosk, sink, is_q=False)
            emit_pending(6)
            acc13 = ps_mm.tile([128, 512], F32, tag="mm")
            for d in range(ND):
                nc.tensor.matmul(acc13, wsl_k[1][:, d, :],
                                 xt[:, d, 3 * 512:4 * 512],
                                 start=(d == 0), stop=(d == ND - 1))
            defer_chain(acc13, kT, 1, 3, cosk, sink, is_q=False)

            def proj_head(h, wsl, out_T, cos_t, sin_t, is_q):
                for c in range(4):
                    acc = ps_mm.tile([128, 512], F32, tag="mm")
                    for d in range(ND):
                        nc.tensor.matmul(acc, wsl[:, d, :],
                                         xt[:, d, c * 512:(c + 1) * 512],
                                         start=(d == 0), stop=(d == ND - 1))
                    emit_pending()
                    defer_chain(acc, out_T, h, c, cos_t, sin_t, is_q)

            for h in (2, 3):
                proj_head(h, wsl_k[h], kT, cosk, sink, is_q=False)

            for tt in range(NTT):
                v_ps = ps_mm.tile([128, 512], F32, tag="mm")
                ts_ = slice(tt * 128, (tt + 1) * 128)
                for d in range(ND):
                    nc.tensor.matmul(v_ps, xt[:, d, ts_], wv_sb[:, d, :],
                                     start=(d == 0), stop=(d == ND - 1))
                emit_pending()
                nc.scalar.copy(v_sb[:, tt, :], v_ps)

            wsl_q = {}

            def get_qslab(h):
                if h < NQH and h not in wsl_q:
                    wsl = wslab_p.tile([128, ND, HD], BF16, tag="wsl")
                    nc.sync.dma_start(out=wsl[:], in_=wq_d[h])
                    wsl_q[h] = wsl

            get_qslab(0)
            get_qslab(1)
            for h in range(NQH):
                get_qslab(h + 2)
                proj_head(h, wsl_q[h], qT, cosq, sinq, is_q=True)
            emit_pending(len(pending))

        # ================= phase 2: attention + out-projection =================
        with ExitStack() as ph2:
            main2 = ph2.enter_context(tc.tile_pool(name="main2", bufs=1))
            # oT split per tq chunk so out-projection reads of chunk-0 rows
            # never serialize against chunk-1 normalize writes (dependency
            # tracking is tile-granular).
            oT_c = [main2.tile([128, NQH, TQC], BF16, tag="oT0", name="oT0"),
                    main2.tile([128, NQH, TQC], BF16, tag="oT1", name="oT1")]
            wo_sb = main2.tile([128, NQH, D], BF16, tag="wo")
            nc.sync.dma_start(out=wo_sb[:], in_=wo_d[:])

            ysb_p = ph2.enter_context(tc.tile_pool(name="ysb", bufs=4))
            pwork = ph2.enter_context(tc.tile_pool(name="ppool", bufs=6))
            cspool = ph2.enter_context(tc.tile_pool(name="cspool", bufs=2))
            awork = ph2.enter_context(tc.tile_pool(name="awork", bufs=2))

            def outproj_mk(tt):
                """Out-projection for t-tile tt as a list of 16 unit closures.

                Each unit is one LDW (oT row-slab) + two 512-col matmuls into
                the np half\'s PSUM pair; the caller separates the np0->np1
                handoff by two slots so the DVE copies recycle ps_y without
                stalling.  y goes to DRAM per 1024-col half.
                """
                oT_src = oT_c[tt // 8]
                ts_ = slice((tt % 8) * 128, (tt % 8 + 1) * 128)
                td_ = slice(tt * 128, (tt + 1) * 128)
                state = {}

                def unit(np_, h):
                    def emit():
                        if h == 0:
                            state[np_] = (
                                ps_y.tile([128, 512], F32, tag="y",
                                          name=f"yp0_{tt}_{np_}"),
                                ps_y.tile([128, 512], F32, tag="y",
                                          name=f"yp1_{tt}_{np_}"))
                        yp0, yp1 = state[np_]
                        ns0 = slice(np_ * 1024, np_ * 1024 + 512)
                        ns1 = slice(np_ * 1024 + 512, (np_ + 1) * 1024)
                        nc.tensor.matmul(yp0, oT_src[:, h, ts_], wo_sb[:, h, ns0],
                                         start=(h == 0), stop=(h == NQH - 1))
                        nc.tensor.matmul(yp1, oT_src[:, h, ts_], wo_sb[:, h, ns1],
                                         start=(h == 0), stop=(h == NQH - 1))
                        if h == NQH - 1:
                            y_sb = ysb_p.tile([128, 1024], F32, tag="ysb")
                            nc.vector.tensor_copy(y_sb[:, 0:512], yp0)
                            nc.vector.tensor_copy(y_sb[:, 512:1024], yp1)
                            nc.sync.dma_start(
                                out=y_d[td_, np_ * 1024:(np_ + 1) * 1024],
                                in_=y_sb[:])
                    return emit

                return [unit(np_, h) for np_ in range(2) for h in range(NQH)]

            def attn(h, c, fillers, prev_drain=None):
                """One head-chunk of attention, software-pipelined.

                Slot tk: two 512-col s-matmuls into a double-buffered
                [128, TQC] PSUM tile, one exp (ACT), one colsum add (DVE);
                o-matmuls lag LAG slots so exp latency never stalls the PE.
                `fillers[slot]` emits independent out-projection units.
                The softmax denominator rides the just-freed final s-tile
                (ones-matmul partition reduction, rows only), then recip on
                DVE and a POOL broadcast -- all off the PE critical path.
                """
                kv = h // 2
                cs0 = slice(c * TQC, c * TQC + 512)
                cs1 = slice(c * TQC + 512, (c + 1) * TQC)
                o_ps = ps_o.tile([128, TQC], F32, tag="o")
                csa = cspool.tile([128, TQC], BF16, tag="csa")
                pend = deque()
                s_ps = None
                den_slot = 2 if c == 0 else 1
                for slot in range(NTT + LAG):
                    if slot == den_slot and prev_drain is not None:
                        prev_drain()
                        prev_drain = None
                    if slot < NTT:
                        tk = slot
                        s_ps = ps_s.tile([128, TQC], F32, tag="s")
                        ks = slice(tk * 128, (tk + 1) * 128)
                        nc.tensor.matmul(s_ps[:, 0:512], kT[:, kv, ks],
                                         qT[:, h, cs0], start=True, stop=True)
                        nc.tensor.matmul(s_ps[:, 512:TQC], kT[:, kv, ks],
                                         qT[:, h, cs1], start=True, stop=True)
                        p_bf = pwork.tile([128, TQC], BF16, tag="p")
                        nc.scalar.activation(out=p_bf, in_=s_ps, func=AF.Exp,
                                             bias=zero128[:, :])
                        if tk == 0:
                            nc.vector.tensor_copy(csa, p_bf)
                        else:
                            nc.vector.tensor_add(csa, csa, p_bf)
                        pend.append((tk, p_bf))
                    for f in fillers.get(slot, ()):
                        f()
                    if slot >= LAG and pend:
                        tk2, p_bf2 = pend.popleft()
                        vt = v_sb[:, tk2, kv * HD:(kv + 1) * HD]
                        nc.tensor.matmul(o_ps[:, 0:512], vt, p_bf2[:, 0:512],
                                         start=(tk2 == 0), stop=(tk2 == NTT - 1))
                        nc.tensor.matmul(o_ps[:, 512:TQC], vt, p_bf2[:, 512:TQC],
                                         start=(tk2 == 0), stop=(tk2 == NTT - 1))
                oTun = awork.tile([128, TQC], BF16, tag="oTun")
                nc.vector.tensor_copy(oTun, o_ps)
                den_ps = s_ps

                def drain():
                    # denominator rows ride the block's final s-tile, whose
                    # buffer the NEXT block only reuses at s(den_slot) --
                    # deferring to that point hides the wait on the last
                    # colsum add without any extra PSUM.
                    nc.tensor.matmul(den_ps[0:1, 0:512], ones_col,
                                     csa[:, 0:512], start=True, stop=True)
                    nc.tensor.matmul(den_ps[0:1, 512:TQC], ones_col,
                                     csa[:, 512:TQC], start=True, stop=True)
                    recr = awork.tile([1, TQC], F32, tag="recr", name="recr")
                    nc.vector.reciprocal_approx_fast(out=recr, in_=den_ps[0:1, :])
                    bc = awork.tile([128, TQC], F32, tag="bc", name="bc")
                    nc.gpsimd.partition_broadcast(bc, recr)
                    nc.vector.tensor_mul(oT_c[c][:, h, :], oTun, bc)

                return drain

            def mk_fillers(units, first_block):
                """Slot map for 16 out-projection units of one t-tile.

                np0 in slots 2..9, np1 in 12..15 (x2) -- the 2-slot gap lets
                the np0 DVE copies recycle ps_y.  For the first chunk-1 block
                everything shifts late so the deferred chunk-0 normalize of
                head 7 (emitted at slot 6) lands before its oT rows are read.
                """
                f = {}
                if first_block:
                    for i in range(4):
                        f[10 + i] = [units[2 * i], units[2 * i + 1]]
                    for i in range(4):
                        f[16 + i] = [units[8 + 2 * i], units[9 + 2 * i]]
                else:
                    for i in range(8):
                        f[2 + i] = [units[i]]
                    for i in range(4):
                        f[12 + i] = [units[8 + 2 * i], units[9 + 2 * i]]
                return f

            # chunk-0 sweep is ACT-exp-paced: with no out-projection PSUM
            # needed yet, a third s buffer decouples s(tk+2) from exp(tk)
            # so the ACT runs back-to-back exps at its streaming floor.
            with ExitStack() as ph2a:
                ps_s = ph2a.enter_context(
                    tc.tile_pool(name="ps_s0", bufs=3, space="PSUM"))
                ps_o = ph2a.enter_context(
                    tc.tile_pool(name="ps_o0", bufs=1, space="PSUM"))
                dr = None
                for h in range(NQH):
                    dr = attn(h, 0, {}, dr)
                dr()  # last chunk-0 drain must stay inside this PSUM scope
            with ExitStack() as ph2a:
                ps_s = ph2a.enter_context(
                    tc.tile_pool(name="ps_s", bufs=2, space="PSUM"))
                ps_o = ph2a.enter_context(
                    tc.tile_pool(name="ps_o", bufs=1, space="PSUM"))
                ps_y = ph2a.enter_context(
                    tc.tile_pool(name="ps_y", bufs=2, space="PSUM"))
                dr = None
                for h in range(NQH):
                    fillers = mk_fillers(outproj_mk(h), first_block=(h == 0))
                    dr = attn(h, 1, fillers, dr)
                dr()

            # tail: t-tiles 8..15 over all heads, 6 rotating PSUM banks
            with ExitStack() as ph2b:
                ps_y = ph2b.enter_context(
                    tc.tile_pool(name="ps_yt", bufs=6, space="PSUM"))
                for tt in range(8, NTT):
                    for f in outproj_mk(tt):
                        f()

    nc.compile()
    return nc


def _get_module():
    if "nc" not in _CACHE:
        _CACHE["nc"] = build_module()
    return _CACHE["nc"]


def _pack_inputs(x, cos, sin, Wq, Wk, Wv, Wo, q_gamma, k_gamma):
    """Host-side prep: per-core input dicts with bf16 packed layouts."""
    bf16 = ml_dtypes.bfloat16
    perm = np.concatenate([np.arange(0, HD, 2), np.arange(1, HD, 2)])  # [128]
    partner = np.concatenate([perm[64:], perm[:64]])                   # gamma idx for sin term
    sign = np.concatenate([-np.ones(64), np.ones(64)]).astype(np.float32)

    cosT = np.ascontiguousarray(cos.T)  # [128, T]
    sinT = np.ascontiguousarray(sin.T)

    def tables(gamma):
        c = (cosT[perm] * gamma[perm][:, None]).astype(bf16)
        s = (sinT[perm] * sign[:, None] * gamma[partner][:, None]).astype(bf16)
        return np.ascontiguousarray(c), np.ascontiguousarray(s)

    cosq, sinq = tables(q_gamma.astype(np.float32))
    cosk, sink = tables(k_gamma.astype(np.float32))

    per_hg = []
    for hg in range(2):
        qh = slice(hg * NQH * HD, (hg + 1) * NQH * HD)
        kh = slice(hg * NKV * HD, (hg + 1) * NKV * HD)
        wq = Wq[:, qh].reshape(ND, 128, NQH, HD)[..., perm]
        wq = np.ascontiguousarray(wq.transpose(2, 1, 0, 3)).astype(bf16)
        wk = Wk[:, kh].reshape(ND, 128, NKV, HD)[..., perm]
        wk = np.ascontiguousarray(wk.transpose(2, 1, 0, 3)).astype(bf16)
        wv = Wv[:, kh].reshape(ND, 128, NKV * HD)
        wv = np.ascontiguousarray(wv.transpose(1, 0, 2)).astype(bf16)
        wo = Wo[hg * NQH * HD:(hg + 1) * NQH * HD, :].reshape(NQH, 128, D)
        wo = np.ascontiguousarray(wo.transpose(1, 0, 2)).astype(bf16)
        per_hg.append(dict(wq=wq, wk=wk, wv=wv, wo=wo))

    in_maps = []
    for b in range(4):
        xt = np.ascontiguousarray(
            x[b].T.reshape(ND, 128, T).transpose(1, 0, 2)).astype(bf16)
        for hg in range(2):
            m = dict(xt=xt, cosq=cosq, sinq=sinq, cosk=cosk, sink=sink,
                     **per_hg[hg])
            in_maps.append(m)
    return in_maps


def kernel(x, cos, sin, Wq, Wk, Wv, Wo, q_gamma, k_gamma, **run_kwargs):
    global LAST_RESULTS
    args = [np.asarray(a, dtype=np.float32)
            for a in (x, cos, sin, Wq, Wk, Wv, Wo, q_gamma, k_gamma)]
    nc = _get_module()
    in_maps = _pack_inputs(*args)
    res = run_bass_kernel_spmd(nc, in_maps, core_ids=list(range(8)), **run_kwargs)
    LAST_RESULTS = res
    y = np.empty((4, T, D), dtype=np.float32)
    for b in range(4):
        y[b] = np.asarray(res.results[2 * b]["y"]) + np.asarray(res.results[2 * b + 1]["y"])
    return y
